# revision 9
# baseline (speedup 1.0000x reference)
"""BLSTM Trainium2 kernel: 8-core SPMD, wavefront schedule.

Core pair q={2q,2q+1} owns batch element q. Even core runs the forward
2-layer LSTM chain, odd core the backward chain (host feeds frames with
both the step axis and the frame order reversed, which makes the device
program parity-free). Per 8-step chunk, the schedule interleaves on one
PE queue: layer-0 recurrence, layer-1 recurrence (1 chunk behind),
the zx input GEMMs for both layers, and the per-stream projection with
overlap-add into a dual accumulator (natural lower half + reversed
upper half). A single pairwise ReduceScatter(add) at the end combines
the two streams; the host flips the odd core's segment.
"""
import numpy as np
from contextlib import ExitStack

U = 512
S = 200          # frame width (LSTM steps)
F = 41           # frames per batch element
T = 4200
STRIDE = 100
HALF = 2100
COLS = S * F     # 8200 device columns, col = s*41 + f
G = 4 * U        # 2048 gate rows
NCORES = 8
KT = U // 128    # 4 k-tiles
MT = G // 128    # 16 m-tiles
CH = 8           # steps per chunk
NCH = S // CH    # 25 chunks

_CACHE = {}


def _build():
    import os
    do_coll = os.environ.get("BL_COLLECTIVE", "1") == "1"
    import concourse.bacc as bacc
    import concourse.tile as tile
    import concourse.bass as bass
    from concourse import mybir
    from concourse.alu_op_type import AluOpType

    f32 = mybir.dt.float32
    f16 = mybir.dt.float16
    AF = mybir.ActivationFunctionType

    nc = bacc.Bacc("TRN2", target_bir_lowering=False, debug=False,
                   num_devices=NCORES)

    xT = nc.dram_tensor("xT", [U, COLS], f16, kind="ExternalInput")
    Wx0 = nc.dram_tensor("Wx0", [U, G], f16, kind="ExternalInput")
    Wh0 = nc.dram_tensor("Wh0", [U, G], f16, kind="ExternalInput")
    Wx1 = nc.dram_tensor("Wx1", [U, G], f16, kind="ExternalInput")
    Wh1 = nc.dram_tensor("Wh1", [U, G], f16, kind="ExternalInput")
    b0d = nc.dram_tensor("b0", [G, 1], f32, kind="ExternalInput")
    b1d = nc.dram_tensor("b1", [G, 1], f32, kind="ExternalInput")
    Wpd = nc.dram_tensor("Wp", [U, U], f16, kind="ExternalInput")
    bpd = nc.dram_tensor("bp", [U, 1], f32, kind="ExternalInput")
    skipd = nc.dram_tensor("skip", [U, HALF], f32, kind="ExternalInput")
    eyed = nc.dram_tensor("eye", [128, 128], f16, kind="ExternalInput")
    outd = nc.dram_tensor("out", [U, HALF], f32, kind="ExternalOutput")

    with ExitStack() as ctx:
        tc = ctx.enter_context(tile.TileContext(nc))
        wpool = ctx.enter_context(tc.tile_pool(name="w", bufs=1))
        accp = ctx.enter_context(tc.tile_pool(name="acc", bufs=1))
        xp = ctx.enter_context(tc.tile_pool(name="x", bufs=2))
        zp = [ctx.enter_context(tc.tile_pool(name=f"z{l}", bufs=2))
              for l in range(2)]
        hp = [ctx.enter_context(tc.tile_pool(name=f"h{l}", bufs=2))
              for l in range(2)]
        gp = [ctx.enter_context(tc.tile_pool(name=f"g{l}", bufs=1))
              for l in range(2)]
        cpools = [ctx.enter_context(tc.tile_pool(name=f"c{l}", bufs=2))
                  for l in range(2)]
        tp = [ctx.enter_context(tc.tile_pool(name=f"t{l}", bufs=1))
              for l in range(2)]
        ptp = ctx.enter_context(tc.tile_pool(name="pt", bufs=2))
        psr = [ctx.enter_context(
            tc.tile_pool(name=f"psr{l}", bufs=1, space="PSUM"))
            for l in range(2)]
        psg = ctx.enter_context(tc.tile_pool(name="psg", bufs=2, space="PSUM"))
        psp = ctx.enter_context(tc.tile_pool(name="psp", bufs=2, space="PSUM"))
        dram = ctx.enter_context(tc.tile_pool(name="dram", bufs=1,
                                              space="DRAM"))

        in_d = dram.tile([2 * U, HALF], f32, name="in_d")
        rs_d = dram.tile([U, HALF], f32, name="rs_d")

        # ---- weights / constants
        def load_w(src, tag, cols):
            tiles = []
            for k in range(KT):
                t = wpool.tile([128, cols], f16, tag=f"{tag}{k}",
                               name=f"w_{tag}{k}")
                nc.sync.dma_start(t[:], src[k * 128:(k + 1) * 128, :])
                tiles.append(t)
            return tiles

        wx = [load_w(Wx0, "wx0", G), load_w(Wx1, "wx1", G)]
        wh = [load_w(Wh0, "wh0", G), load_w(Wh1, "wh1", G)]
        wp = load_w(Wpd, "wp", U)

        eye = wpool.tile([128, 128], f16, tag="eye")
        nc.sync.dma_start(eye[:], eyed[:])

        b0t = wpool.tile([128, MT], f32, tag="b0")
        nc.sync.dma_start(b0t[:], b0d[:].rearrange("(m p) o -> p (m o)", p=128))
        b1t = wpool.tile([128, MT], f32, tag="b1")
        nc.sync.dma_start(b1t[:], b1d[:].rearrange("(m p) o -> p (m o)", p=128))
        bt = [b0t, b1t]
        bpt = wpool.tile([128, KT], f32, tag="bp")
        nc.sync.dma_start(bpt[:], bpd[:].rearrange("(m p) o -> p (m o)", p=128))

        # ---- accumulators: accA = skip-initialized lower half (natural),
        # accB = upper half in reversed device time
        accA = accp.tile([128, KT, HALF], f32, tag="accA")
        nc.sync.dma_start(accA[:], skipd[:].rearrange("(k p) c -> p k c", p=128))
        accB = accp.tile([128, KT, HALF], f32, tag="accB")
        nc.vector.memset(accB[:, 0:2, :], 0.0)
        nc.gpsimd.memset(accB[:, 2:4, :], 0.0)
        accBr = accB[:, :, ::-1]

        xr = xT[:].rearrange("(k p) c -> p k c", p=128)
        xt = [None] * NCH
        zt = [[None] * NCH for _ in range(2)]
        ht = [[None] * NCH for _ in range(2)]
        cst = [None, None]

        def emit_X(c):
            xt[c] = xp.tile([128, KT, CH, F], f16, tag="x", name=f"x{c}")
            nc.sync.dma_start(
                xt[c][:].rearrange("p k s f -> p k (s f)"),
                xr[:, :, c * CH * F:(c + 1) * CH * F])

        def emit_G(l, c, m_lo, m_hi):
            # zx GEMM for layer l, chunk c, m-tiles [m_lo, m_hi)
            src = xt[c] if l == 0 else ht[0][c]
            if m_lo == 0:
                zt[l][c] = zp[l].tile([128, MT, CH, F], f16, tag="z", name=f"z{l}_{c}")
            z = zt[l][c]
            for m in range(m_lo, m_hi):
                ps = psg.tile([128, CH * F], f32, tag="ps")
                for k in range(KT):
                    nc.tensor.matmul(ps[:], wx[l][k][:, m * 128:(m + 1) * 128],
                                     src[:, k, :, :],
                                     start=(k == 0), stop=(k == KT - 1))
                # PSUM->SBUF move + bias, spread across engines to keep
                # the Act engine free for the recurrence activations
                if l == 0:
                    nc.vector.tensor_scalar_add(z[:, m, :, :], ps[:],
                                                bt[l][:, m:m + 1])
                elif m < 8:
                    nc.scalar.activation(z[:, m, :, :], ps[:], AF.Identity,
                                         bias=bt[l][:, m:m + 1])
                else:
                    nc.vector.tensor_scalar_add(z[:, m, :, :], ps[:],
                                                bt[l][:, m:m + 1])

        def emit_rec_step(l, s):
            c, si = divmod(s, CH)
            if si == 0:
                ht[l][c] = hp[l].tile([128, KT, CH, F], f16, tag="h", name=f"h{l}_{c}")
            z = zt[l][c]
            ps_if = psr[l].tile([128, 8 * F], f32, tag="if")
            ps_go = psr[l].tile([128, 8 * F], f32, tag="go")
            nc.tensor.matmul(ps_if[:], eye[:], z[:, 0:8, si, :],
                             start=True, stop=(s == 0))
            nc.tensor.matmul(ps_go[:], eye[:], z[:, 8:16, si, :],
                             start=True, stop=(s == 0))
            if s > 0:
                hc, hsi = ((ht[l][c - 1], CH - 1) if si == 0
                           else (ht[l][c], si - 1))
                for ps, m_lo in ((ps_if, 0), (ps_go, 8)):
                    for mi in range(8):
                        m = m_lo + mi
                        for k in range(KT):
                            nc.tensor.matmul(
                                ps[:, mi * F:(mi + 1) * F],
                                wh[l][k][:, m * 128:(m + 1) * 128],
                                hc[:, k, hsi, :],
                                start=False,
                                stop=(mi == 7 and k == KT - 1),
                                skip_group_check=True)
            sif = gp[l].tile([128, 8 * F], f32, tag="sif")
            nc.scalar.activation(sif[:], ps_if[:], AF.Sigmoid)
            sgo = gp[l].tile([128, 8 * F], f32, tag="sgo")
            nc.scalar.activation(sgo[:, 0:4 * F], ps_go[:, 0:4 * F], AF.Tanh)
            nc.scalar.activation(sgo[:, 4 * F:], ps_go[:, 4 * F:], AF.Sigmoid)
            cnew = cpools[l].tile([128, 4 * F], f32, tag="c")
            if s == 0:
                nc.vector.tensor_mul(cnew[:], sif[:, 0:4 * F], sgo[:, 0:4 * F])
            else:
                t1 = tp[l].tile([128, 4 * F], f32, tag="t1")
                nc.vector.tensor_mul(t1[:], sif[:, 0:4 * F], sgo[:, 0:4 * F])
                t2 = tp[l].tile([128, 4 * F], f32, tag="t2")
                nc.gpsimd.tensor_mul(t2[:], sif[:, 4 * F:], cst[l][:])
                nc.vector.tensor_add(cnew[:], t1[:], t2[:])
            th = tp[l].tile([128, 4 * F], f32, tag="th")
            nc.scalar.activation(th[:], cnew[:], AF.Tanh)
            nc.gpsimd.tensor_mul(ht[l][c][:, :, si, :], sgo[:, 4 * F:], th[:])
            cst[l] = cnew

        def emit_P(c):
            pt = ptp.tile([128, KT, CH, F], f16, tag="pt")
            for m in range(KT):
                ps = psp.tile([128, CH * F], f32, tag="ps")
                for k in range(KT):
                    nc.tensor.matmul(ps[:], wp[k][:, m * 128:(m + 1) * 128],
                                     ht[1][c][:, k, :, :],
                                     start=(k == 0), stop=(k == KT - 1))
                nc.scalar.activation(pt[:, m, :, :], ps[:], AF.Identity,
                                     bias=bpt[:, m:m + 1])
            for si in range(CH):
                sg = c * CH + si
                cntA = 21 if sg < 100 else 20
                endA = sg + (cntA - 1) * 100 + 1
                nc.vector.tensor_add(accA[:, :, sg:endA:100],
                                     accA[:, :, sg:endA:100],
                                     pt[:, :, si, 0:cntA])
                cntB = F - cntA
                base = cntA * 100 + sg - HALF
                endB = base + (cntB - 1) * 100 + 1
                nc.gpsimd.tensor_add(accBr[:, :, base:endB:100],
                                     accBr[:, :, base:endB:100],
                                     pt[:, :, si, cntA:F])

        # ---- wavefront
        emit_X(0)
        emit_X(1)
        emit_G(0, 0, 0, MT)
        for c in range(NCH):
            if c + 2 < NCH:
                emit_X(c + 2)
            for si in range(CH):
                emit_rec_step(0, c * CH + si)
                if c >= 1:
                    emit_rec_step(1, (c - 1) * CH + si)
                if c + 1 < NCH:
                    emit_G(0, c + 1, 2 * si, 2 * si + 2)
            emit_G(1, c, 0, MT)
            if c >= 1:
                emit_P(c - 1)
        for si in range(CH):
            emit_rec_step(1, (NCH - 1) * CH + si)
        emit_P(NCH - 1)

        # ---- pairwise exchange: my rank's block gets accA (my half),
        # partner's block gets accB (their half, already in their coords)
        pid = nc.partition_id()
        rank = nc.s_assert_within(pid % 2, 0, 1, skip_runtime_assert=True)
        other = nc.s_assert_within(1 - pid % 2, 0, 1, skip_runtime_assert=True)
        in_r = in_d[:].rearrange("(b k p) c -> p b k c", p=128, k=KT)
        nc.sync.dma_start(in_r[:, bass.ds(rank, 1)], accA[:])
        nc.sync.dma_start(in_r[:, bass.ds(other, 1)], accB[:])
        if do_coll:
            nc.gpsimd.collective_compute(
                "ReduceScatter", AluOpType.add,
                replica_groups=[[0, 1], [2, 3], [4, 5], [6, 7]],
                ins=[in_d[:]], outs=[rs_d[:]])
        else:
            nc.sync.dma_start(rs_d[:], in_d[U:2 * U, :])
        nc.sync.dma_start(outd[:], rs_d[:])

    nc.compile()
    return nc


def _prep_inputs(inputs, Wx_f0, Wh_f0, b_f0, Wx_f1, Wh_f1, b_f1,
                 Wx_b0, Wh_b0, b_b0, Wx_b1, Wh_b1, b_b1, Wp, bp):
    x = np.asarray(inputs, dtype=np.float32)  # [4, 512, 4200]
    eye = np.eye(128, dtype=np.float16)
    idx = np.arange(F)[:, None] * STRIDE + np.arange(S)[None, :]  # [F, S]
    wsets = {
        0: (Wx_f0, Wh_f0, b_f0, Wx_f1, Wh_f1, b_f1),
        1: (Wx_b0, Wh_b0, b_b0, Wx_b1, Wh_b1, b_b1),
    }
    Wp = np.asarray(Wp)
    bph = (np.asarray(bp, np.float32) * 0.5).reshape(U, 1)
    in_maps = []
    for core in range(NCORES):
        q, par = core // 2, core % 2
        xs = x[q][:, idx]                       # [U, F, S]
        if par:
            xs = xs[:, ::-1, ::-1]
            skip = np.ascontiguousarray(x[q][:, HALF:][:, ::-1])
            Wp_own = Wp[U:]
        else:
            skip = np.ascontiguousarray(x[q][:, :HALF])
            Wp_own = Wp[:U]
        xdev = np.ascontiguousarray(
            xs.transpose(0, 2, 1).reshape(U, COLS)).astype(np.float16)
        wx0, wh0, b0, wx1, wh1, b1 = wsets[par]
        in_maps.append({
            "xT": xdev,
            "Wx0": np.asarray(wx0, np.float16),
            "Wh0": np.asarray(wh0, np.float16),
            "Wx1": np.asarray(wx1, np.float16),
            "Wh1": np.asarray(wh1, np.float16),
            "b0": np.asarray(b0, np.float32).reshape(G, 1),
            "b1": np.asarray(b1, np.float32).reshape(G, 1),
            "Wp": np.asarray(Wp_own, np.float16),
            "bp": bph,
            "skip": skip,
            "eye": eye,
        })
    return in_maps


def kernel(**inputs) -> np.ndarray:
    from concourse.bass_utils import run_bass_kernel_spmd

    if "nc" not in _CACHE:
        _CACHE["nc"] = _build()
    nc = _CACHE["nc"]

    import os
    in_maps = _prep_inputs(**inputs)
    trace = os.environ.get("BL_TRACE", "0") == "1"
    res = run_bass_kernel_spmd(nc, in_maps, list(range(NCORES)), trace=trace)
    _CACHE["last_result"] = res

    out = np.zeros((4, U, T), dtype=np.float32)
    for core in range(NCORES):
        q, par = core // 2, core % 2
        seg = res.results[core]["out"]  # [U, HALF]
        if par == 0:
            out[q][:, :HALF] = seg
        else:
            out[q][:, HALF:] = seg[:, ::-1]
    return out


# revision 12
# speedup vs baseline: 1.1559x; 1.1559x over previous
"""BLSTM Trainium2 kernel: 8-core SPMD, wavefront schedule.

Core pair q={2q,2q+1} owns batch element q. Even core runs the forward
2-layer LSTM chain, odd core the backward chain (host feeds frames with
both the step axis and the frame order reversed, which makes the device
program parity-free). Per 8-step chunk, the schedule interleaves on one
PE queue: layer-0 recurrence, layer-1 recurrence (1 chunk behind),
the zx input GEMMs for both layers, and the per-stream projection with
overlap-add into a dual accumulator (natural lower half + reversed
upper half). A single pairwise ReduceScatter(add) at the end combines
the two streams; the host flips the odd core's segment.
"""
import numpy as np
from contextlib import ExitStack

U = 512
S = 200          # frame width (LSTM steps)
F = 41           # frames per batch element
T = 4200
STRIDE = 100
HALF = 2100
COLS = S * F     # 8200 device columns, col = s*41 + f
G = 4 * U        # 2048 gate rows
NCORES = 8
KT = U // 128    # 4 k-tiles
MT = G // 128    # 16 m-tiles
CH = 8           # steps per chunk
NCH = S // CH    # 25 chunks

_CACHE = {}


def _build():
    import os
    do_coll = os.environ.get("BL_COLLECTIVE", "1") == "1"
    import concourse.bacc as bacc
    import concourse.tile as tile
    import concourse.bass as bass
    from concourse import mybir
    from concourse.alu_op_type import AluOpType

    f32 = mybir.dt.float32
    f16 = mybir.dt.float16
    AF = mybir.ActivationFunctionType

    nc = bacc.Bacc("TRN2", target_bir_lowering=False, debug=False,
                   num_devices=NCORES)

    xT = nc.dram_tensor("xT", [U, COLS], f16, kind="ExternalInput")
    Wx0 = nc.dram_tensor("Wx0", [U, G], f16, kind="ExternalInput")
    Wh0 = nc.dram_tensor("Wh0", [U, G], f16, kind="ExternalInput")
    Wx1 = nc.dram_tensor("Wx1", [U, G], f16, kind="ExternalInput")
    Wh1 = nc.dram_tensor("Wh1", [U, G], f16, kind="ExternalInput")
    b0d = nc.dram_tensor("b0", [G, 1], f32, kind="ExternalInput")
    b1d = nc.dram_tensor("b1", [G, 1], f32, kind="ExternalInput")
    Wpd = nc.dram_tensor("Wp", [U, U], f16, kind="ExternalInput")
    bpd = nc.dram_tensor("bp", [U, 1], f32, kind="ExternalInput")
    skipd = nc.dram_tensor("skip", [U, HALF], f16, kind="ExternalInput")
    eyed = nc.dram_tensor("eye", [128, 128], f16, kind="ExternalInput")
    outd = nc.dram_tensor("out", [U, HALF], f16, kind="ExternalOutput")

    with ExitStack() as ctx:
        tc = ctx.enter_context(tile.TileContext(nc))
        wpool = ctx.enter_context(tc.tile_pool(name="w", bufs=1))
        accp = ctx.enter_context(tc.tile_pool(name="acc", bufs=1))
        xp = ctx.enter_context(tc.tile_pool(name="x", bufs=2))
        zp = [ctx.enter_context(tc.tile_pool(name=f"z{l}", bufs=2))
              for l in range(2)]
        hp = [ctx.enter_context(tc.tile_pool(name=f"h{l}", bufs=2))
              for l in range(2)]
        gp = [ctx.enter_context(tc.tile_pool(name=f"g{l}", bufs=2))
              for l in range(2)]
        cpools = [ctx.enter_context(tc.tile_pool(name=f"c{l}", bufs=2))
                  for l in range(2)]
        tp = [ctx.enter_context(tc.tile_pool(name=f"t{l}", bufs=2))
              for l in range(2)]
        ptp = ctx.enter_context(tc.tile_pool(name="pt", bufs=2))
        psr = [ctx.enter_context(
            tc.tile_pool(name=f"psr{l}", bufs=1, space="PSUM"))
            for l in range(2)]
        psg = ctx.enter_context(tc.tile_pool(name="psg", bufs=2, space="PSUM"))
        psp = ctx.enter_context(tc.tile_pool(name="psp", bufs=2, space="PSUM"))
        dram = ctx.enter_context(tc.tile_pool(name="dram", bufs=1,
                                              space="DRAM"))

        in_d = dram.tile([2 * U, HALF], f16, name="in_d")
        rs_d = dram.tile([U, HALF], f16, name="rs_d")

        # ---- weights / constants
        def load_w(src, tag, cols):
            tiles = []
            for k in range(KT):
                t = wpool.tile([128, cols], f16, tag=f"{tag}{k}",
                               name=f"w_{tag}{k}")
                nc.sync.dma_start(t[:], src[k * 128:(k + 1) * 128, :])
                tiles.append(t)
            return tiles

        wx = [load_w(Wx0, "wx0", G), None]
        b0t = wpool.tile([128, MT], f32, tag="b0")
        nc.sync.dma_start(b0t[:], b0d[:].rearrange("(m p) o -> p (m o)", p=128))
        eye = wpool.tile([128, 128], f16, tag="eye")
        nc.sync.dma_start(eye[:], eyed[:])
        bt = [b0t, None]

        xr = xT[:].rearrange("(k p) c -> p k c", p=128)
        xt = [None] * NCH
        zt = [[None] * NCH for _ in range(2)]
        ht = [[None] * NCH for _ in range(2)]
        cst = [None, None]

        def emit_X(c):
            xt[c] = xp.tile([128, KT, CH, F], f16, tag="x", name=f"x{c}")
            nc.sync.dma_start(
                xt[c][:].rearrange("p k s f -> p k (s f)"),
                xr[:, :, c * CH * F:(c + 1) * CH * F])

        def emit_G(l, c, m_lo, m_hi):
            # zx GEMM for layer l, chunk c, m-tiles [m_lo, m_hi)
            src = xt[c] if l == 0 else ht[0][c]
            if m_lo == 0:
                zt[l][c] = zp[l].tile([128, MT, CH, F], f16, tag="z", name=f"z{l}_{c}")
            z = zt[l][c]
            for m in range(m_lo, m_hi):
                ps = psg.tile([128, CH * F], f32, tag="ps")
                for k in range(KT):
                    nc.tensor.matmul(ps[:], wx[l][k][:, m * 128:(m + 1) * 128],
                                     src[:, k, :, :],
                                     start=(k == 0), stop=(k == KT - 1))
                nc.scalar.activation(z[:, m, :, :], ps[:], AF.Identity,
                                     bias=bt[l][:, m:m + 1])

        def emit_rec_step(l, s):
            c, si = divmod(s, CH)
            if si == 0:
                ht[l][c] = hp[l].tile([128, KT, CH, F], f16, tag="h", name=f"h{l}_{c}")
            z = zt[l][c]
            ps_if = psr[l].tile([128, 8 * F], f32, tag="if")
            ps_go = psr[l].tile([128, 8 * F], f32, tag="go")
            nc.tensor.matmul(ps_if[:], eye[:], z[:, 0:8, si, :],
                             start=True, stop=(s == 0))
            nc.tensor.matmul(ps_go[:], eye[:], z[:, 8:16, si, :],
                             start=True, stop=(s == 0))
            if s > 0:
                hc, hsi = ((ht[l][c - 1], CH - 1) if si == 0
                           else (ht[l][c], si - 1))
                for ps, m_lo in ((ps_if, 0), (ps_go, 8)):
                    for mi in range(8):
                        m = m_lo + mi
                        for k in range(KT):
                            nc.tensor.matmul(
                                ps[:, mi * F:(mi + 1) * F],
                                wh[l][k][:, m * 128:(m + 1) * 128],
                                hc[:, k, hsi, :],
                                start=False,
                                stop=(mi == 7 and k == KT - 1),
                                skip_group_check=True)
            sif = gp[l].tile([128, 8 * F], f32, tag="sif")
            nc.scalar.activation(sif[:], ps_if[:], AF.Sigmoid)
            sgo = gp[l].tile([128, 8 * F], f32, tag="sgo")
            nc.scalar.activation(sgo[:, 0:4 * F], ps_go[:, 0:4 * F], AF.Tanh)
            nc.scalar.activation(sgo[:, 4 * F:], ps_go[:, 4 * F:], AF.Sigmoid)
            cnew = cpools[l].tile([128, 4 * F], f32, tag="c")
            if s == 0:
                nc.vector.tensor_mul(cnew[:], sif[:, 0:4 * F], sgo[:, 0:4 * F])
            else:
                t1 = tp[l].tile([128, 4 * F], f32, tag="t1")
                nc.vector.tensor_mul(t1[:], sif[:, 0:4 * F], sgo[:, 0:4 * F])
                t2 = tp[l].tile([128, 4 * F], f32, tag="t2")
                nc.gpsimd.tensor_mul(t2[:], sif[:, 4 * F:], cst[l][:])
                nc.vector.tensor_add(cnew[:], t1[:], t2[:])
            th = tp[l].tile([128, 4 * F], f32, tag="th")
            nc.scalar.activation(th[:], cnew[:], AF.Tanh)
            nc.gpsimd.tensor_mul(ht[l][c][:, :, si, :], sgo[:, 4 * F:], th[:])
            cst[l] = cnew

        def emit_P(c):
            pt = ptp.tile([128, KT, CH, F], f16, tag="pt")
            for m in range(KT):
                ps = psp.tile([128, CH * F], f32, tag="ps")
                for k in range(KT):
                    nc.tensor.matmul(ps[:], wp[k][:, m * 128:(m + 1) * 128],
                                     ht[1][c][:, k, :, :],
                                     start=(k == 0), stop=(k == KT - 1))
                nc.scalar.activation(pt[:, m, :, :], ps[:], AF.Identity,
                                     bias=bpt[:, m:m + 1])
            for si in range(CH):
                sg = c * CH + si
                cntA = 21 if sg < 100 else 20
                endA = sg + (cntA - 1) * 100 + 1
                nc.vector.tensor_add(accA[:, :, sg:endA:100],
                                     accA[:, :, sg:endA:100],
                                     pt[:, :, si, 0:cntA])
                cntB = F - cntA
                base = cntA * 100 + sg - HALF
                endB = base + (cntB - 1) * 100 + 1
                nc.gpsimd.tensor_add(accBr[:, :, base:endB:100],
                                     accBr[:, :, base:endB:100],
                                     pt[:, :, si, cntA:F])

        # ---- wavefront (late-load everything not needed by G0(0))
        emit_X(0)
        emit_X(1)
        emit_G(0, 0, 0, MT)
        wh = [load_w(Wh0, "wh0", G), load_w(Wh1, "wh1", G)]
        wx[1] = load_w(Wx1, "wx1", G)
        wp = load_w(Wpd, "wp", U)
        b1t = wpool.tile([128, MT], f32, tag="b1")
        nc.sync.dma_start(b1t[:], b1d[:].rearrange("(m p) o -> p (m o)", p=128))
        bt[1] = b1t
        bpt = wpool.tile([128, KT], f32, tag="bp")
        nc.sync.dma_start(bpt[:], bpd[:].rearrange("(m p) o -> p (m o)", p=128))
        accA = accp.tile([128, KT, HALF], f16, tag="accA")
        nc.sync.dma_start(accA[:], skipd[:].rearrange("(k p) c -> p k c", p=128))
        accB = accp.tile([128, KT, HALF], f16, tag="accB")
        nc.vector.memset(accB[:, 0:2, :], 0.0)
        nc.gpsimd.memset(accB[:, 2:4, :], 0.0)
        accBr = accB[:, :, ::-1]
        for c in range(NCH):
            if c + 2 < NCH:
                emit_X(c + 2)
            for si in range(CH):
                emit_rec_step(0, c * CH + si)
                if c >= 1:
                    emit_rec_step(1, (c - 1) * CH + si)
                if c + 1 < NCH:
                    emit_G(0, c + 1, 2 * si, 2 * si + 2)
            emit_G(1, c, 0, MT)
            if c >= 1:
                emit_P(c - 1)
        for si in range(CH):
            emit_rec_step(1, (NCH - 1) * CH + si)
        emit_P(NCH - 1)

        # ---- pairwise exchange: my rank's block gets accA (my half),
        # partner's block gets accB (their half, already in their coords)
        pid = nc.partition_id()
        rank = nc.s_assert_within(pid % 2, 0, 1, skip_runtime_assert=True)
        other = nc.s_assert_within(1 - pid % 2, 0, 1, skip_runtime_assert=True)
        in_r = in_d[:].rearrange("(b k p) c -> p b k c", p=128, k=KT)
        nc.sync.dma_start(in_r[:, bass.ds(rank, 1)], accA[:])
        nc.sync.dma_start(in_r[:, bass.ds(other, 1)], accB[:])
        if do_coll:
            nc.gpsimd.collective_compute(
                "ReduceScatter", AluOpType.add,
                replica_groups=[[0, 1], [2, 3], [4, 5], [6, 7]],
                ins=[in_d[:]], outs=[rs_d[:]])
        else:
            nc.sync.dma_start(rs_d[:], in_d[U:2 * U, :])
        nc.sync.dma_start(outd[:], rs_d[:])

    nc.compile()
    return nc


def _prep_inputs(inputs, Wx_f0, Wh_f0, b_f0, Wx_f1, Wh_f1, b_f1,
                 Wx_b0, Wh_b0, b_b0, Wx_b1, Wh_b1, b_b1, Wp, bp):
    x = np.asarray(inputs, dtype=np.float32)  # [4, 512, 4200]
    eye = np.eye(128, dtype=np.float16)
    idx = np.arange(F)[:, None] * STRIDE + np.arange(S)[None, :]  # [F, S]
    wsets = {
        0: (Wx_f0, Wh_f0, b_f0, Wx_f1, Wh_f1, b_f1),
        1: (Wx_b0, Wh_b0, b_b0, Wx_b1, Wh_b1, b_b1),
    }
    Wp = np.asarray(Wp)
    bph = (np.asarray(bp, np.float32) * 0.5).reshape(U, 1)
    in_maps = []
    for core in range(NCORES):
        q, par = core // 2, core % 2
        xs = x[q][:, idx]                       # [U, F, S]
        if par:
            xs = xs[:, ::-1, ::-1]
            skip = np.ascontiguousarray(x[q][:, HALF:][:, ::-1]).astype(np.float16)
            Wp_own = Wp[U:]
        else:
            skip = np.ascontiguousarray(x[q][:, :HALF]).astype(np.float16)
            Wp_own = Wp[:U]
        xdev = np.ascontiguousarray(
            xs.transpose(0, 2, 1).reshape(U, COLS)).astype(np.float16)
        wx0, wh0, b0, wx1, wh1, b1 = wsets[par]
        in_maps.append({
            "xT": xdev,
            "Wx0": np.asarray(wx0, np.float16),
            "Wh0": np.asarray(wh0, np.float16),
            "Wx1": np.asarray(wx1, np.float16),
            "Wh1": np.asarray(wh1, np.float16),
            "b0": np.asarray(b0, np.float32).reshape(G, 1),
            "b1": np.asarray(b1, np.float32).reshape(G, 1),
            "Wp": np.asarray(Wp_own, np.float16),
            "bp": bph,
            "skip": skip,
            "eye": eye,
        })
    return in_maps


def kernel(**inputs) -> np.ndarray:
    from concourse.bass_utils import run_bass_kernel_spmd

    if "nc" not in _CACHE:
        _CACHE["nc"] = _build()
    nc = _CACHE["nc"]

    import os
    in_maps = _prep_inputs(**inputs)
    trace = os.environ.get("BL_TRACE", "0") == "1"
    res = run_bass_kernel_spmd(nc, in_maps, list(range(NCORES)), trace=trace)
    _CACHE["last_result"] = res

    out = np.zeros((4, U, T), dtype=np.float32)
    for core in range(NCORES):
        q, par = core // 2, core % 2
        seg = res.results[core]["out"]  # [U, HALF]
        if par == 0:
            out[q][:, :HALF] = seg
        else:
            out[q][:, HALF:] = seg[:, ::-1]
    return out


# revision 13
# speedup vs baseline: 1.1781x; 1.0192x over previous
"""BLSTM Trainium2 kernel: 8-core SPMD, wavefront schedule.

Core pair q={2q,2q+1} owns batch element q. Even core runs the forward
2-layer LSTM chain, odd core the backward chain (host feeds frames with
both the step axis and the frame order reversed, which makes the device
program parity-free). Per 8-step chunk, the schedule interleaves on one
PE queue: layer-0 recurrence, layer-1 recurrence (1 chunk behind),
the zx input GEMMs for both layers, and the per-stream projection with
overlap-add into a dual accumulator (natural lower half + reversed
upper half). A single pairwise ReduceScatter(add) at the end combines
the two streams; the host flips the odd core's segment.
"""
import numpy as np
import ml_dtypes
from contextlib import ExitStack

U = 512
S = 200          # frame width (LSTM steps)
F = 41           # frames per batch element
T = 4200
STRIDE = 100
HALF = 2100
COLS = S * F     # 8200 device columns, col = s*41 + f
G = 4 * U        # 2048 gate rows
NCORES = 8
KT = U // 128    # 4 k-tiles
MT = G // 128    # 16 m-tiles
CH = 8           # steps per chunk
NCH = S // CH    # 25 chunks

_CACHE = {}


def _build():
    import os
    do_coll = os.environ.get("BL_COLLECTIVE", "1") == "1"
    import concourse.bacc as bacc
    import concourse.tile as tile
    import concourse.bass as bass
    from concourse import mybir
    from concourse.alu_op_type import AluOpType

    f32 = mybir.dt.float32
    f16 = mybir.dt.float16
    f8 = mybir.dt.float8e4
    PM = mybir.MatmulPerfMode
    AF = mybir.ActivationFunctionType

    nc = bacc.Bacc("TRN2", target_bir_lowering=False, debug=False,
                   num_devices=NCORES)

    xT = nc.dram_tensor("xT", [U, COLS], f8, kind="ExternalInput")
    Wx0 = nc.dram_tensor("Wx0", [128, KT, G], f8, kind="ExternalInput")
    Wh0 = nc.dram_tensor("Wh0", [U, G], f16, kind="ExternalInput")
    Wx1 = nc.dram_tensor("Wx1", [U, G], f16, kind="ExternalInput")
    Wh1 = nc.dram_tensor("Wh1", [U, G], f16, kind="ExternalInput")
    b0d = nc.dram_tensor("b0", [G, 1], f32, kind="ExternalInput")
    b1d = nc.dram_tensor("b1", [G, 1], f32, kind="ExternalInput")
    Wpd = nc.dram_tensor("Wp", [U, U], f16, kind="ExternalInput")
    bpd = nc.dram_tensor("bp", [U, 1], f32, kind="ExternalInput")
    skipd = nc.dram_tensor("skip", [U, HALF], f16, kind="ExternalInput")
    eyed = nc.dram_tensor("eye", [128, 128], f16, kind="ExternalInput")
    outd = nc.dram_tensor("out", [U, HALF], f16, kind="ExternalOutput")

    with ExitStack() as ctx:
        tc = ctx.enter_context(tile.TileContext(nc))
        wpool = ctx.enter_context(tc.tile_pool(name="w", bufs=1))
        accp = ctx.enter_context(tc.tile_pool(name="acc", bufs=1))
        xp = ctx.enter_context(tc.tile_pool(name="x", bufs=2))
        zp = [ctx.enter_context(tc.tile_pool(name=f"z{l}", bufs=2))
              for l in range(2)]
        hp = [ctx.enter_context(tc.tile_pool(name=f"h{l}", bufs=2))
              for l in range(2)]
        gp = [ctx.enter_context(tc.tile_pool(name=f"g{l}", bufs=2))
              for l in range(2)]
        cpools = [ctx.enter_context(tc.tile_pool(name=f"c{l}", bufs=2))
                  for l in range(2)]
        tp = [ctx.enter_context(tc.tile_pool(name=f"t{l}", bufs=2))
              for l in range(2)]
        ptp = ctx.enter_context(tc.tile_pool(name="pt", bufs=2))
        psr = [ctx.enter_context(
            tc.tile_pool(name=f"psr{l}", bufs=1, space="PSUM"))
            for l in range(2)]
        psg = ctx.enter_context(tc.tile_pool(name="psg", bufs=2, space="PSUM"))
        psp = ctx.enter_context(tc.tile_pool(name="psp", bufs=2, space="PSUM"))
        dram = ctx.enter_context(tc.tile_pool(name="dram", bufs=1,
                                              space="DRAM"))

        in_d = dram.tile([2 * U, HALF], f16, name="in_d")
        rs_d = dram.tile([U, HALF], f16, name="rs_d")

        # ---- weights / constants
        def load_w(src, tag, cols):
            tiles = []
            for k in range(KT):
                t = wpool.tile([128, cols], f16, tag=f"{tag}{k}",
                               name=f"w_{tag}{k}")
                nc.sync.dma_start(t[:], src[k * 128:(k + 1) * 128, :])
                tiles.append(t)
            return tiles

        wx0t = wpool.tile([128, KT, G], f8, tag="wx0")
        nc.sync.dma_start(wx0t[:], Wx0[:])
        wx = [wx0t, None]
        b0t = wpool.tile([128, MT], f32, tag="b0")
        nc.sync.dma_start(b0t[:], b0d[:].rearrange("(m p) o -> p (m o)", p=128))
        eye = wpool.tile([128, 128], f16, tag="eye")
        nc.sync.dma_start(eye[:], eyed[:])
        bt = [b0t, None]

        xr = xT[:].rearrange("(k p) c -> p k c", p=128)
        xt = [None] * NCH
        zt = [[None] * NCH for _ in range(2)]
        ht = [[None] * NCH for _ in range(2)]
        cst = [None, None]

        def emit_X(c):
            xt[c] = xp.tile([128, KT, CH, F], f8, tag="x", name=f"x{c}")
            nc.sync.dma_start(
                xt[c][:].rearrange("p k s f -> p k (s f)"),
                xr[:, :, c * CH * F:(c + 1) * CH * F])

        def emit_G(l, c, m_lo, m_hi):
            # zx GEMM for layer l, chunk c, m-tiles [m_lo, m_hi)
            src = xt[c] if l == 0 else ht[0][c]
            if m_lo == 0:
                zt[l][c] = zp[l].tile([128, MT, CH, F], f16, tag="z", name=f"z{l}_{c}")
            z = zt[l][c]
            for m in range(m_lo, m_hi):
                ps = psg.tile([128, CH * F], f32, tag="ps")
                if l == 0:
                    # fp8 DoubleRow: 2 k-subtiles per matmul, weights x16
                    for j in range(2):
                        nc.tensor.matmul(
                            ps[:],
                            wx[0][:, 2 * j:2 * j + 2, m * 128:(m + 1) * 128],
                            src[:, 2 * j:2 * j + 2, :, :],
                            start=(j == 0), stop=(j == 1),
                            perf_mode=PM.DoubleRow)
                    nc.scalar.activation(z[:, m, :, :], ps[:], AF.Identity,
                                         bias=bt[l][:, m:m + 1],
                                         scale=1.0 / 16.0)
                else:
                    for k in range(KT):
                        nc.tensor.matmul(ps[:],
                                         wx[l][k][:, m * 128:(m + 1) * 128],
                                         src[:, k, :, :],
                                         start=(k == 0), stop=(k == KT - 1))
                    nc.scalar.activation(z[:, m, :, :], ps[:], AF.Identity,
                                         bias=bt[l][:, m:m + 1])

        def emit_rec_step(l, s):
            c, si = divmod(s, CH)
            if si == 0:
                ht[l][c] = hp[l].tile([128, KT, CH, F], f16, tag="h", name=f"h{l}_{c}")
            z = zt[l][c]
            ps_if = psr[l].tile([128, 8 * F], f32, tag="if")
            ps_go = psr[l].tile([128, 8 * F], f32, tag="go")
            nc.tensor.matmul(ps_if[:], eye[:], z[:, 0:8, si, :],
                             start=True, stop=(s == 0))
            nc.tensor.matmul(ps_go[:], eye[:], z[:, 8:16, si, :],
                             start=True, stop=(s == 0))
            if s > 0:
                hc, hsi = ((ht[l][c - 1], CH - 1) if si == 0
                           else (ht[l][c], si - 1))
                for ps, m_lo in ((ps_if, 0), (ps_go, 8)):
                    for mi in range(8):
                        m = m_lo + mi
                        for k in range(KT):
                            nc.tensor.matmul(
                                ps[:, mi * F:(mi + 1) * F],
                                wh[l][k][:, m * 128:(m + 1) * 128],
                                hc[:, k, hsi, :],
                                start=False,
                                stop=(mi == 7 and k == KT - 1),
                                skip_group_check=True)
            sif = gp[l].tile([128, 8 * F], f32, tag="sif")
            nc.scalar.activation(sif[:], ps_if[:], AF.Sigmoid)
            sgo = gp[l].tile([128, 8 * F], f32, tag="sgo")
            nc.scalar.activation(sgo[:, 0:4 * F], ps_go[:, 0:4 * F], AF.Tanh)
            nc.scalar.activation(sgo[:, 4 * F:], ps_go[:, 4 * F:], AF.Sigmoid)
            cnew = cpools[l].tile([128, 4 * F], f32, tag="c")
            if s == 0:
                nc.vector.tensor_mul(cnew[:], sif[:, 0:4 * F], sgo[:, 0:4 * F])
            else:
                t1 = tp[l].tile([128, 4 * F], f32, tag="t1")
                nc.vector.tensor_mul(t1[:], sif[:, 0:4 * F], sgo[:, 0:4 * F])
                t2 = tp[l].tile([128, 4 * F], f32, tag="t2")
                nc.gpsimd.tensor_mul(t2[:], sif[:, 4 * F:], cst[l][:])
                nc.vector.tensor_add(cnew[:], t1[:], t2[:])
            th = tp[l].tile([128, 4 * F], f32, tag="th")
            nc.scalar.activation(th[:], cnew[:], AF.Tanh)
            nc.gpsimd.tensor_mul(ht[l][c][:, :, si, :], sgo[:, 4 * F:], th[:])
            cst[l] = cnew

        def emit_P(c):
            pt = ptp.tile([128, KT, CH, F], f16, tag="pt")
            for m in range(KT):
                ps = psp.tile([128, CH * F], f32, tag="ps")
                for k in range(KT):
                    nc.tensor.matmul(ps[:], wp[k][:, m * 128:(m + 1) * 128],
                                     ht[1][c][:, k, :, :],
                                     start=(k == 0), stop=(k == KT - 1))
                nc.scalar.activation(pt[:, m, :, :], ps[:], AF.Identity,
                                     bias=bpt[:, m:m + 1])
            for si in range(CH):
                sg = c * CH + si
                cntA = 21 if sg < 100 else 20
                endA = sg + (cntA - 1) * 100 + 1
                nc.vector.tensor_add(accA[:, :, sg:endA:100],
                                     accA[:, :, sg:endA:100],
                                     pt[:, :, si, 0:cntA])
                cntB = F - cntA
                base = cntA * 100 + sg - HALF
                endB = base + (cntB - 1) * 100 + 1
                nc.gpsimd.tensor_add(accBr[:, :, base:endB:100],
                                     accBr[:, :, base:endB:100],
                                     pt[:, :, si, cntA:F])

        # ---- wavefront (late-load everything not needed by G0(0))
        emit_X(0)
        emit_X(1)
        emit_G(0, 0, 0, MT)
        wh = [load_w(Wh0, "wh0", G), load_w(Wh1, "wh1", G)]
        wx[1] = load_w(Wx1, "wx1", G)
        wp = load_w(Wpd, "wp", U)
        b1t = wpool.tile([128, MT], f32, tag="b1")
        nc.sync.dma_start(b1t[:], b1d[:].rearrange("(m p) o -> p (m o)", p=128))
        bt[1] = b1t
        bpt = wpool.tile([128, KT], f32, tag="bp")
        nc.sync.dma_start(bpt[:], bpd[:].rearrange("(m p) o -> p (m o)", p=128))
        accA = accp.tile([128, KT, HALF], f16, tag="accA")
        nc.sync.dma_start(accA[:], skipd[:].rearrange("(k p) c -> p k c", p=128))
        accB = accp.tile([128, KT, HALF], f16, tag="accB")
        nc.vector.memset(accB[:, 0:2, :], 0.0)
        nc.gpsimd.memset(accB[:, 2:4, :], 0.0)
        accBr = accB[:, :, ::-1]
        for c in range(NCH):
            if c + 2 < NCH:
                emit_X(c + 2)
            for si in range(CH):
                emit_rec_step(0, c * CH + si)
                if c >= 1:
                    emit_rec_step(1, (c - 1) * CH + si)
                if c + 1 < NCH:
                    emit_G(0, c + 1, 2 * si, 2 * si + 2)
            emit_G(1, c, 0, MT)
            if c >= 1:
                emit_P(c - 1)
        for si in range(CH):
            emit_rec_step(1, (NCH - 1) * CH + si)
        emit_P(NCH - 1)

        # ---- pairwise exchange: my rank's block gets accA (my half),
        # partner's block gets accB (their half, already in their coords)
        pid = nc.partition_id()
        rank = nc.s_assert_within(pid % 2, 0, 1, skip_runtime_assert=True)
        other = nc.s_assert_within(1 - pid % 2, 0, 1, skip_runtime_assert=True)
        in_r = in_d[:].rearrange("(b k p) c -> p b k c", p=128, k=KT)
        nc.sync.dma_start(in_r[:, bass.ds(rank, 1)], accA[:])
        nc.sync.dma_start(in_r[:, bass.ds(other, 1)], accB[:])
        if do_coll:
            nc.gpsimd.collective_compute(
                "ReduceScatter", AluOpType.add,
                replica_groups=[[0, 1], [2, 3], [4, 5], [6, 7]],
                ins=[in_d[:]], outs=[rs_d[:]])
        else:
            nc.sync.dma_start(rs_d[:], in_d[U:2 * U, :])
        nc.sync.dma_start(outd[:], rs_d[:])

    nc.compile()
    return nc


def _prep_inputs(inputs, Wx_f0, Wh_f0, b_f0, Wx_f1, Wh_f1, b_f1,
                 Wx_b0, Wh_b0, b_b0, Wx_b1, Wh_b1, b_b1, Wp, bp):
    x = np.asarray(inputs, dtype=np.float32)  # [4, 512, 4200]
    eye = np.eye(128, dtype=np.float16)
    idx = np.arange(F)[:, None] * STRIDE + np.arange(S)[None, :]  # [F, S]
    wsets = {
        0: (Wx_f0, Wh_f0, b_f0, Wx_f1, Wh_f1, b_f1),
        1: (Wx_b0, Wh_b0, b_b0, Wx_b1, Wh_b1, b_b1),
    }
    Wp = np.asarray(Wp)
    bph = (np.asarray(bp, np.float32) * 0.5).reshape(U, 1)
    in_maps = []
    for core in range(NCORES):
        q, par = core // 2, core % 2
        xs = x[q][:, idx]                       # [U, F, S]
        if par:
            xs = xs[:, ::-1, ::-1]
            skip = np.ascontiguousarray(x[q][:, HALF:][:, ::-1]).astype(np.float16)
            Wp_own = Wp[U:]
        else:
            skip = np.ascontiguousarray(x[q][:, :HALF]).astype(np.float16)
            Wp_own = Wp[:U]
        xdev = np.ascontiguousarray(
            xs.transpose(0, 2, 1).reshape(U, COLS)).astype(ml_dtypes.float8_e4m3)
        wx0, wh0, b0, wx1, wh1, b1 = wsets[par]
        in_maps.append({
            "xT": xdev,
            "Wx0": np.ascontiguousarray(
                (np.asarray(wx0, np.float32) * 16.0)
                .reshape(KT, 128, G).transpose(1, 0, 2)
            ).astype(ml_dtypes.float8_e4m3),
            "Wh0": np.asarray(wh0, np.float16),
            "Wx1": np.asarray(wx1, np.float16),
            "Wh1": np.asarray(wh1, np.float16),
            "b0": np.asarray(b0, np.float32).reshape(G, 1),
            "b1": np.asarray(b1, np.float32).reshape(G, 1),
            "Wp": np.asarray(Wp_own, np.float16),
            "bp": bph,
            "skip": skip,
            "eye": eye,
        })
    return in_maps


def kernel(**inputs) -> np.ndarray:
    from concourse.bass_utils import run_bass_kernel_spmd

    if "nc" not in _CACHE:
        _CACHE["nc"] = _build()
    nc = _CACHE["nc"]

    import os
    in_maps = _prep_inputs(**inputs)
    trace = os.environ.get("BL_TRACE", "0") == "1"
    res = run_bass_kernel_spmd(nc, in_maps, list(range(NCORES)), trace=trace)
    _CACHE["last_result"] = res

    out = np.zeros((4, U, T), dtype=np.float32)
    for core in range(NCORES):
        q, par = core // 2, core % 2
        seg = res.results[core]["out"]  # [U, HALF]
        if par == 0:
            out[q][:, :HALF] = seg
        else:
            out[q][:, HALF:] = seg[:, ::-1]
    return out


# revision 14
# speedup vs baseline: 1.2433x; 1.0553x over previous
"""BLSTM Trainium2 kernel: 8-core SPMD, wavefront schedule.

Core pair q={2q,2q+1} owns batch element q. Even core runs the forward
2-layer LSTM chain, odd core the backward chain (host feeds frames with
both the step axis and the frame order reversed, which makes the device
program parity-free). Per 8-step chunk, the schedule interleaves on one
PE queue: layer-0 recurrence, layer-1 recurrence (1 chunk behind),
the zx input GEMMs for both layers, and the per-stream projection with
overlap-add into a dual accumulator (natural lower half + reversed
upper half). A single pairwise ReduceScatter(add) at the end combines
the two streams; the host flips the odd core's segment.
"""
import numpy as np
import ml_dtypes
from contextlib import ExitStack

U = 512
S = 200          # frame width (LSTM steps)
F = 41           # frames per batch element
T = 4200
STRIDE = 100
HALF = 2100
COLS = S * F     # 8200 device columns, col = s*41 + f
G = 4 * U        # 2048 gate rows
NCORES = 8
KT = U // 128    # 4 k-tiles
MT = G // 128    # 16 m-tiles
CH = 8           # steps per chunk
NCH = S // CH    # 25 chunks

_CACHE = {}


def _build():
    import os
    do_coll = os.environ.get("BL_COLLECTIVE", "1") == "1"
    import concourse.bacc as bacc
    import concourse.tile as tile
    import concourse.bass as bass
    from concourse import mybir
    from concourse.alu_op_type import AluOpType

    f32 = mybir.dt.float32
    f16 = mybir.dt.float16
    f8 = mybir.dt.float8e4
    PM = mybir.MatmulPerfMode
    AF = mybir.ActivationFunctionType

    nc = bacc.Bacc("TRN2", target_bir_lowering=False, debug=False,
                   num_devices=NCORES)

    xT = nc.dram_tensor("xT", [U, COLS], f8, kind="ExternalInput")
    Wx0 = nc.dram_tensor("Wx0", [128, KT, G], f8, kind="ExternalInput")
    Wh0 = nc.dram_tensor("Wh0", [128, KT, G], f8, kind="ExternalInput")
    Wx1 = nc.dram_tensor("Wx1", [128, KT, G], f8, kind="ExternalInput")
    Wh1 = nc.dram_tensor("Wh1", [128, KT, G], f8, kind="ExternalInput")
    b0d = nc.dram_tensor("b0", [G, 1], f32, kind="ExternalInput")
    b1d = nc.dram_tensor("b1", [G, 1], f32, kind="ExternalInput")
    Wpd = nc.dram_tensor("Wp", [128, KT, U], f8, kind="ExternalInput")
    bpd = nc.dram_tensor("bp", [U, 1], f32, kind="ExternalInput")
    skipd = nc.dram_tensor("skip", [U, HALF], f16, kind="ExternalInput")
    eyed = nc.dram_tensor("eye", [128, 128], f16, kind="ExternalInput")
    outd = nc.dram_tensor("out", [U, HALF], f16, kind="ExternalOutput")

    with ExitStack() as ctx:
        tc = ctx.enter_context(tile.TileContext(nc))
        wpool = ctx.enter_context(tc.tile_pool(name="w", bufs=1))
        accp = ctx.enter_context(tc.tile_pool(name="acc", bufs=1))
        xp = ctx.enter_context(tc.tile_pool(name="x", bufs=2))
        zp = [ctx.enter_context(tc.tile_pool(name=f"z{l}", bufs=2))
              for l in range(2)]
        hp = [ctx.enter_context(tc.tile_pool(name=f"h{l}", bufs=2))
              for l in range(2)]
        gp = [ctx.enter_context(tc.tile_pool(name=f"g{l}", bufs=2))
              for l in range(2)]
        cpools = [ctx.enter_context(tc.tile_pool(name=f"c{l}", bufs=2))
                  for l in range(2)]
        tp = [ctx.enter_context(tc.tile_pool(name=f"t{l}", bufs=2))
              for l in range(2)]
        ptp = ctx.enter_context(tc.tile_pool(name="pt", bufs=2))
        psr = [ctx.enter_context(
            tc.tile_pool(name=f"psr{l}", bufs=1, space="PSUM"))
            for l in range(2)]
        psg = ctx.enter_context(tc.tile_pool(name="psg", bufs=2, space="PSUM"))
        psp = ctx.enter_context(tc.tile_pool(name="psp", bufs=2, space="PSUM"))
        dram = ctx.enter_context(tc.tile_pool(name="dram", bufs=1,
                                              space="DRAM"))

        in_d = dram.tile([2 * U, HALF], f16, name="in_d")
        rs_d = dram.tile([U, HALF], f16, name="rs_d")

        # ---- weights / constants
        def load_w(src, tag, cols):
            tiles = []
            for k in range(KT):
                t = wpool.tile([128, cols], f16, tag=f"{tag}{k}",
                               name=f"w_{tag}{k}")
                nc.sync.dma_start(t[:], src[k * 128:(k + 1) * 128, :])
                tiles.append(t)
            return tiles

        def load_w8(srcd, tag, cols):
            t = wpool.tile([128, KT, cols], f8, tag=tag, name=f"w_{tag}")
            nc.sync.dma_start(t[:], srcd[:])
            return t

        wx = [load_w8(Wx0, "wx0", G), None]
        b0t = wpool.tile([128, MT], f32, tag="b0")
        nc.sync.dma_start(b0t[:], b0d[:].rearrange("(m p) o -> p (m o)", p=128))
        eye = wpool.tile([128, 128], f16, tag="eye")
        nc.sync.dma_start(eye[:], eyed[:])
        bt = [b0t, None]

        xr = xT[:].rearrange("(k p) c -> p k c", p=128)
        xt = [None] * NCH
        zt = [[None] * NCH for _ in range(2)]
        ht = [[None] * NCH for _ in range(2)]
        cst = [None, None]

        def emit_X(c):
            xt[c] = xp.tile([128, KT, CH, F], f8, tag="x", name=f"x{c}")
            nc.sync.dma_start(
                xt[c][:].rearrange("p k s f -> p k (s f)"),
                xr[:, :, c * CH * F:(c + 1) * CH * F])

        def emit_G(l, c, m_lo, m_hi):
            # zx GEMM for layer l, chunk c, m-tiles [m_lo, m_hi)
            src = xt[c] if l == 0 else ht[0][c]
            if m_lo == 0:
                zt[l][c] = zp[l].tile([128, MT, CH, F], f16, tag="z", name=f"z{l}_{c}")
            z = zt[l][c]
            for m in range(m_lo, m_hi):
                ps = psg.tile([128, CH * F], f32, tag="ps")
                # fp8 DoubleRow: 2 k-subtiles per matmul, weights x16
                for j in range(2):
                    nc.tensor.matmul(
                        ps[:],
                        wx[l][:, 2 * j:2 * j + 2, m * 128:(m + 1) * 128],
                        src[:, 2 * j:2 * j + 2, :, :],
                        start=(j == 0), stop=(j == 1),
                        perf_mode=PM.DoubleRow)
                if l == 0:
                    nc.vector.tensor_scalar(z[:, m, :, :], ps[:],
                                            1.0 / 16.0, bt[l][:, m:m + 1],
                                            AluOpType.mult, AluOpType.add)
                else:
                    nc.scalar.activation(z[:, m, :, :], ps[:], AF.Identity,
                                         bias=bt[l][:, m:m + 1],
                                         scale=1.0 / 16.0)

        def emit_rec_step(l, s):
            c, si = divmod(s, CH)
            if si == 0:
                ht[l][c] = hp[l].tile([128, KT, CH, F], f8, tag="h", name=f"h{l}_{c}")
            z = zt[l][c]
            ps_if = psr[l].tile([128, 8 * F], f32, tag="if")
            ps_go = psr[l].tile([128, 8 * F], f32, tag="go")
            nc.tensor.matmul(ps_if[:], eye[:], z[:, 0:8, si, :],
                             start=True, stop=(s == 0))
            nc.tensor.matmul(ps_go[:], eye[:], z[:, 8:16, si, :],
                             start=True, stop=(s == 0))
            if s > 0:
                hc, hsi = ((ht[l][c - 1], CH - 1) if si == 0
                           else (ht[l][c], si - 1))
                for ps, m_lo in ((ps_if, 0), (ps_go, 8)):
                    for mi in range(8):
                        m = m_lo + mi
                        for j in range(2):
                            nc.tensor.matmul(
                                ps[:, mi * F:(mi + 1) * F],
                                wh[l][:, 2 * j:2 * j + 2,
                                      m * 128:(m + 1) * 128],
                                hc[:, 2 * j:2 * j + 2, hsi, :],
                                start=False,
                                stop=(mi == 7 and j == 1),
                                skip_group_check=True,
                                perf_mode=PM.DoubleRow)
            sif = gp[l].tile([128, 8 * F], f32, tag="sif")
            nc.scalar.activation(sif[:], ps_if[:], AF.Sigmoid,
                                 scale=1.0 / 16.0)
            sgo = gp[l].tile([128, 8 * F], f32, tag="sgo")
            nc.scalar.activation(sgo[:, 0:4 * F], ps_go[:, 0:4 * F], AF.Tanh,
                                 scale=1.0 / 16.0)
            nc.scalar.activation(sgo[:, 4 * F:], ps_go[:, 4 * F:], AF.Sigmoid,
                                 scale=1.0 / 16.0)
            cnew = cpools[l].tile([128, 4 * F], f32, tag="c")
            if s == 0:
                nc.vector.tensor_mul(cnew[:], sif[:, 0:4 * F], sgo[:, 0:4 * F])
            else:
                t1 = tp[l].tile([128, 4 * F], f32, tag="t1")
                nc.vector.tensor_mul(t1[:], sif[:, 0:4 * F], sgo[:, 0:4 * F])
                t2 = tp[l].tile([128, 4 * F], f32, tag="t2")
                nc.gpsimd.tensor_mul(t2[:], sif[:, 4 * F:], cst[l][:])
                nc.vector.tensor_add(cnew[:], t1[:], t2[:])
            th = tp[l].tile([128, 4 * F], f32, tag="th")
            nc.scalar.activation(th[:], cnew[:], AF.Tanh)
            nc.gpsimd.tensor_mul(ht[l][c][:, :, si, :], sgo[:, 4 * F:], th[:])
            cst[l] = cnew

        def emit_P(c):
            pt = ptp.tile([128, KT, CH, F], f16, tag="pt")
            for m in range(KT):
                ps = psp.tile([128, CH * F], f32, tag="ps")
                for j in range(2):
                    nc.tensor.matmul(
                        ps[:], wp[:, 2 * j:2 * j + 2, m * 128:(m + 1) * 128],
                        ht[1][c][:, 2 * j:2 * j + 2, :, :],
                        start=(j == 0), stop=(j == 1),
                        perf_mode=PM.DoubleRow)
                nc.scalar.activation(pt[:, m, :, :], ps[:], AF.Identity,
                                     bias=bpt[:, m:m + 1], scale=1.0 / 16.0)
            for si in range(CH):
                sg = c * CH + si
                cntA = 21 if sg < 100 else 20
                endA = sg + (cntA - 1) * 100 + 1
                nc.vector.tensor_add(accA[:, :, sg:endA:100],
                                     accA[:, :, sg:endA:100],
                                     pt[:, :, si, 0:cntA])
                cntB = F - cntA
                base = cntA * 100 + sg - HALF
                endB = base + (cntB - 1) * 100 + 1
                nc.gpsimd.tensor_add(accBr[:, :, base:endB:100],
                                     accBr[:, :, base:endB:100],
                                     pt[:, :, si, cntA:F])

        # ---- wavefront (late-load everything not needed by G0(0))
        emit_X(0)
        emit_X(1)
        emit_G(0, 0, 0, MT)
        wh = [load_w8(Wh0, "wh0", G), load_w8(Wh1, "wh1", G)]
        wx[1] = load_w8(Wx1, "wx1", G)
        wp = load_w8(Wpd, "wp", U)
        b1t = wpool.tile([128, MT], f32, tag="b1")
        nc.sync.dma_start(b1t[:], b1d[:].rearrange("(m p) o -> p (m o)", p=128))
        bt[1] = b1t
        bpt = wpool.tile([128, KT], f32, tag="bp")
        nc.sync.dma_start(bpt[:], bpd[:].rearrange("(m p) o -> p (m o)", p=128))
        accA = accp.tile([128, KT, HALF], f16, tag="accA")
        nc.sync.dma_start(accA[:], skipd[:].rearrange("(k p) c -> p k c", p=128))
        accB = accp.tile([128, KT, HALF], f16, tag="accB")
        nc.vector.memset(accB[:, 0:2, :], 0.0)
        nc.gpsimd.memset(accB[:, 2:4, :], 0.0)
        accBr = accB[:, :, ::-1]
        for c in range(NCH):
            if c + 2 < NCH:
                emit_X(c + 2)
            for si in range(CH):
                emit_rec_step(0, c * CH + si)
                if c >= 1:
                    emit_rec_step(1, (c - 1) * CH + si)
                if c + 1 < NCH:
                    emit_G(0, c + 1, 2 * si, 2 * si + 2)
            emit_G(1, c, 0, MT)
            if c >= 1:
                emit_P(c - 1)
        for si in range(CH):
            emit_rec_step(1, (NCH - 1) * CH + si)
        emit_P(NCH - 1)

        # ---- pairwise exchange: my rank's block gets accA (my half),
        # partner's block gets accB (their half, already in their coords)
        pid = nc.partition_id()
        rank = nc.s_assert_within(pid % 2, 0, 1, skip_runtime_assert=True)
        other = nc.s_assert_within(1 - pid % 2, 0, 1, skip_runtime_assert=True)
        in_r = in_d[:].rearrange("(b k p) c -> p b k c", p=128, k=KT)
        nc.sync.dma_start(in_r[:, bass.ds(rank, 1)], accA[:])
        nc.sync.dma_start(in_r[:, bass.ds(other, 1)], accB[:])
        if do_coll:
            nc.gpsimd.collective_compute(
                "ReduceScatter", AluOpType.add,
                replica_groups=[[0, 1], [2, 3], [4, 5], [6, 7]],
                ins=[in_d[:]], outs=[rs_d[:]])
        else:
            nc.sync.dma_start(rs_d[:], in_d[U:2 * U, :])
        nc.sync.dma_start(outd[:], rs_d[:])

    nc.compile()
    return nc


def _prep_inputs(inputs, Wx_f0, Wh_f0, b_f0, Wx_f1, Wh_f1, b_f1,
                 Wx_b0, Wh_b0, b_b0, Wx_b1, Wh_b1, b_b1, Wp, bp):
    x = np.asarray(inputs, dtype=np.float32)  # [4, 512, 4200]
    eye = (np.eye(128) * 16.0).astype(np.float16)
    idx = np.arange(F)[:, None] * STRIDE + np.arange(S)[None, :]  # [F, S]
    wsets = {
        0: (Wx_f0, Wh_f0, b_f0, Wx_f1, Wh_f1, b_f1),
        1: (Wx_b0, Wh_b0, b_b0, Wx_b1, Wh_b1, b_b1),
    }
    Wp = np.asarray(Wp)
    bph = (np.asarray(bp, np.float32) * 0.5).reshape(U, 1)

    def pack8(w, cols):
        return np.ascontiguousarray(
            (np.asarray(w, np.float32) * 16.0)
            .reshape(KT, 128, cols).transpose(1, 0, 2)
        ).astype(ml_dtypes.float8_e4m3)
    in_maps = []
    for core in range(NCORES):
        q, par = core // 2, core % 2
        xs = x[q][:, idx]                       # [U, F, S]
        if par:
            xs = xs[:, ::-1, ::-1]
            skip = np.ascontiguousarray(x[q][:, HALF:][:, ::-1]).astype(np.float16)
            Wp_own = Wp[U:]
        else:
            skip = np.ascontiguousarray(x[q][:, :HALF]).astype(np.float16)
            Wp_own = Wp[:U]
        xdev = np.ascontiguousarray(
            xs.transpose(0, 2, 1).reshape(U, COLS)).astype(ml_dtypes.float8_e4m3)
        wx0, wh0, b0, wx1, wh1, b1 = wsets[par]
        in_maps.append({
            "xT": xdev,
            "Wx0": pack8(wx0, G),
            "Wh0": pack8(wh0, G),
            "Wx1": pack8(wx1, G),
            "Wh1": pack8(wh1, G),
            "b0": np.asarray(b0, np.float32).reshape(G, 1),
            "b1": np.asarray(b1, np.float32).reshape(G, 1),
            "Wp": pack8(Wp_own, U),
            "bp": bph,
            "skip": skip,
            "eye": eye,
        })
    return in_maps


def kernel(**inputs) -> np.ndarray:
    from concourse.bass_utils import run_bass_kernel_spmd

    if "nc" not in _CACHE:
        _CACHE["nc"] = _build()
    nc = _CACHE["nc"]

    import os
    in_maps = _prep_inputs(**inputs)
    trace = os.environ.get("BL_TRACE", "0") == "1"
    res = run_bass_kernel_spmd(nc, in_maps, list(range(NCORES)), trace=trace)
    _CACHE["last_result"] = res

    out = np.zeros((4, U, T), dtype=np.float32)
    for core in range(NCORES):
        q, par = core // 2, core % 2
        seg = res.results[core]["out"]  # [U, HALF]
        if par == 0:
            out[q][:, :HALF] = seg
        else:
            out[q][:, HALF:] = seg[:, ::-1]
    return out
